# revision 1
# baseline (speedup 1.0000x reference)
"""ChunkMHSA (banded local-window attention) Trainium2 kernel.

Full-input contract: kernel(**inputs) takes the complete tensors from
setup_inputs() and returns the full [B, T, D] output.  Internally the
sequence dimension is sharded 8 ways (256 queries per NeuronCore) with a
front/back halo of 6/3 tokens, so each core runs the whole fused pipeline
(LayerNorm -> QKV -> banded softmax(QK^T)V -> output projection ->
residual) independently -- no collectives.

Per-core dataflow (SPMD, one Bass program):
  x[tok,D] f32 --bn_stats--> mean/rstd --ts--> xr f16 --PE transpose-->
  xTr[D,tok] --PE f16 matmuls--> q,k [hk,tok] and vT [tok,hk]
  scores psum[q,s] = mask + q.k ; ACT exp(scale=1/8, accum sums) ;
  normalize on DVE ; PE transpose -> attnT[s,q] ; ctx[hk,q] = vT.T@attnT ;
  out psum[q,D] = ctx.Wo ; ACT evac ; GpSimd residual add ; DMA out.
"""

import os

os.environ.setdefault("JAX_PLATFORMS", "axon")

from contextlib import ExitStack

import numpy as np

import concourse.bass as bass
import concourse.bacc as bacc
import concourse.tile as tile
from concourse import mybir
from concourse.bass_utils import run_bass_kernel_spmd

F32 = mybir.dt.float32
F16 = mybir.dt.float16

B, T, D = 2, 2048, 512
H, DH = 8, 64
WF, WB = 6, 3
LN_EPS = 1e-3
NCORES = 8
TLOC = T // NCORES          # 256 queries per core
TIN = WF + TLOC + WB        # 265 local tokens incl. halo
NTT = 3                     # token tiles per batch (128+128+9)
NQC = 2                     # query chunks of 128 per batch
S = 128 + WF + WB           # 137 keys per query chunk
NEG = -30000.0              # additive mask value (fp16-safe)

_CACHE = {}


def _build_program():
    nc = bacc.Bacc(
        "TRN2", target_bir_lowering=False, debug=False, num_devices=NCORES
    )

    xs = nc.dram_tensor("xs", [B, TIN, D], F32, kind="ExternalInput").ap()
    wall = nc.dram_tensor("wall", [16, 128, D], F16, kind="ExternalInput").ap()
    maskd = nc.dram_tensor("maskd", [NQC, 128, S], F16, kind="ExternalInput").ap()
    eye16d = nc.dram_tensor("eye16", [128, 128], F16, kind="ExternalInput").ap()
    xq32d = nc.dram_tensor("xq32", [B, NQC, 128, D], F32, kind="ExternalInput").ap()
    outd = nc.dram_tensor("out", [B, TLOC, D], F32, kind="ExternalOutput").ap()

    with tile.TileContext(nc) as tc, ExitStack() as ctx:
        _emit(ctx, tc, xs, wall, maskd, eye16d, xq32d, outd)

    nc.compile()
    return nc


def _emit(ctx, tc, xs, wall, maskd, eye16d, xq32d, outd):
    nc = tc.nc
    EXP = mybir.ActivationFunctionType.Exp
    SQRT = mybir.ActivationFunctionType.Sqrt
    COPY = mybir.ActivationFunctionType.Copy
    SUB = mybir.AluOpType.subtract
    MULT = mybir.AluOpType.mult

    consts = ctx.enter_context(tc.tile_pool(name="consts", bufs=1))
    persist = ctx.enter_context(tc.tile_pool(name="persist", bufs=1))
    ln_tmp = ctx.enter_context(tc.tile_pool(name="ln_tmp", bufs=3))
    xr_pool = ctx.enter_context(tc.tile_pool(name="xr", bufs=3))
    attn_tmp = ctx.enter_context(tc.tile_pool(name="attn_tmp", bufs=6))
    # PSUM budget 8 banks: scx(2) proj(2) atm(2) att(1) ctx2(1)
    ps_scx = ctx.enter_context(tc.tile_pool(name="ps_scx", bufs=3, space="PSUM"))
    ps_proj = ctx.enter_context(tc.tile_pool(name="ps_proj", bufs=2, space="PSUM"))
    ps_at = ctx.enter_context(tc.tile_pool(name="ps_at", bufs=1, space="PSUM"))
    ps_att = ctx.enter_context(tc.tile_pool(name="ps_att", bufs=1, space="PSUM"))
    ps_ctx = ctx.enter_context(tc.tile_pool(name="ps_ctx", bufs=1, space="PSUM"))

    # ---- constants / weights (DMA issue spread across idle queues) ----------
    eye16 = consts.tile([128, 128], F16)
    nc.scalar.dma_start(eye16, eye16d)
    xq32 = consts.tile([128, B * NQC, D], F32)
    nc.gpsimd.dma_start(xq32, xq32d.rearrange("b c p d -> p (b c) d"))
    mask_sb = consts.tile([128, NQC, S], F16)
    nc.scalar.dma_start(mask_sb, maskd.rearrange("c p s -> p c s"))
    epst = consts.tile([128, 1], F32)
    nc.vector.memset(epst, LN_EPS)
    # warm the ACT tables (Sqrt/Exp/Copy) during the DMA prologue so the
    # ~1.5us lazy table loads don't land mid-pipeline
    warm = consts.tile([128, 1], F32)
    nc.scalar.activation(out=warm, in_=epst, func=SQRT, bias=epst)
    nc.scalar.activation(out=warm, in_=warm, func=EXP)
    nc.scalar.activation(out=warm, in_=warm, func=COPY)

    # all weights in one DMA: wall[4*widx + j] = chunk j of matrix widx
    w_all = consts.tile([128, 16, D], F16)
    nc.gpsimd.dma_start(w_all, wall.rearrange("m p d -> p m d"))

    def w(name, j):
        widx = "qkvo".index(name)
        return w_all[:, 4 * widx + j, :]

    # ---- x load + LayerNorm + centered/scaled xr + transpose ----------------
    x_sb = persist.tile([128, 2 * NTT, D], F32)
    xtr = persist.tile([128, 4, 2 * 384], F16)   # [dpart, dchunk, b*384+tok]
    q_sb = persist.tile([128, 4, B, TLOC], F16, tag="q_sb")
    k_sb = persist.tile([128, 4, B, TIN], F16, tag="k_sb")
    vt_sb = persist.tile([128, B, NTT, D], F16, tag="vt_sb")
    out_stage = persist.tile([128, B * NQC, D], F32, tag="out_stage")

    for b in range(B):
        nc.gpsimd.memset(x_sb[:, b * NTT + 2, :], 0.0)
    # per-tile x loads so LayerNorm can start on tile 0 early
    for b in range(B):
        eng = nc.sync if b == 0 else nc.scalar
        for i in range(2):
            eng.dma_start(
                x_sb[:, b * NTT + i, :], xs[b, 128 * i : 128 * (i + 1), :]
            )
        eng.dma_start(x_sb[:9, b * NTT + 2, :], xs[b, 256:TIN, :])

    def warm_pe(n, pool, tag, shape):
        fill = pool.tile(shape, F16, tag=tag)
        out = fill[0:9, 0, :].bitcast(F32)
        for _ in range(n):
            nc.tensor.matmul(
                out, eye16[:, 0:9], eye16[:, 0:64], start=True, stop=True
            )

    def emit_ln(b, i):
        xt = x_sb[:, b * NTT + i, :]
        st = ln_tmp.tile([128, 6], F32, tag="st")
        mv = ln_tmp.tile([128, 2], F32, tag="mv")
        nc.vector.bn_stats(out=st, in_=xt)
        nc.vector.bn_aggr(out=mv, in_=st)
        sd = ln_tmp.tile([128, 1], F32, tag="sd")
        nc.scalar.activation(out=sd, in_=mv[:, 1:2], func=SQRT, bias=epst)
        rstd = ln_tmp.tile([128, 1], F32, tag="rstd")
        nc.vector.reciprocal(out=rstd, in_=sd)
        xr = xr_pool.tile([128, D], F16, tag="xr")
        nc.vector.tensor_scalar(
            out=xr, in0=xt, scalar1=mv[:, 0:1], scalar2=rstd,
            op0=SUB, op1=MULT,
        )
        pt = ps_scx.tile([128, 4, 128], F16, tag="scx")
        for j in range(4):
            nc.tensor.transpose(pt[:, j, :], xr[:, 128 * j : 128 * j + 128], eye16)
        nc.scalar.activation(
            out=xtr[:, :, 384 * b + 128 * i : 384 * b + 128 * (i + 1)],
            in_=pt, func=COPY,
        )

    def emit_proj(b):
        # q: queries only (N=256)
        for hkt in range(4):
            pp = ps_proj.tile([128, D], F32, tag="proj")
            for j in range(4):
                nc.tensor.matmul(
                    pp[:, 0:TLOC],
                    w("q", j)[:, 128 * hkt : 128 * (hkt + 1)],
                    xtr[:, j, 384 * b + WF : 384 * b + WF + TLOC],
                    start=(j == 0), stop=(j == 3),
                )
            nc.vector.tensor_copy(q_sb[:, hkt, b, :], pp[:, 0:TLOC])
        # k incl. halo (N=265)
        for hkt in range(4):
            pp = ps_proj.tile([128, D], F32, tag="proj")
            for j in range(4):
                nc.tensor.matmul(
                    pp[:, 0:TIN],
                    w("k", j)[:, 128 * hkt : 128 * (hkt + 1)],
                    xtr[:, j, 384 * b : 384 * b + TIN],
                    start=(j == 0), stop=(j == 3),
                )
            nc.scalar.activation(out=k_sb[:, hkt, b, :], in_=pp[:, 0:TIN], func=COPY)
        # vT per token tile (N=512)
        for i in range(NTT):
            pp = ps_proj.tile([128, D], F32, tag="proj")
            for j in range(4):
                nc.tensor.matmul(
                    pp,
                    xtr[:, j, 384 * b + 128 * i : 384 * b + 128 * (i + 1)],
                    w("v", j),
                    start=(j == 0), stop=(j == 3),
                )
            if i % 2 == 0:
                nc.scalar.activation(out=vt_sb[:, b, i, :], in_=pp, func=COPY)
            else:
                nc.vector.tensor_copy(vt_sb[:, b, i, :], pp)

    def emit_attn(b, cq):
        q0 = 128 * cq
        s0 = 128 * cq
        at_m = ps_at.tile([128, 8, 128], F16, tag="atm")
        at_t = ps_att.tile([9, 8, 128], F16, tag="att")
        ctx2 = ps_ctx.tile([128, 4, 128], F32, tag="ctx2")
        for h in range(8):
            hp = 64 * (h % 2)
            hkt = h // 2
            sc = ps_scx.tile([128, S], F32, tag="scx")
            nc.tensor.matmul(sc, eye16, mask_sb[:, cq, :], start=True, stop=False)
            nc.tensor.matmul(
                sc,
                q_sb[hp : hp + 64, hkt, b, q0 : q0 + 128],
                k_sb[hp : hp + 64, hkt, b, s0 : s0 + S],
                start=False, stop=True,
            )
            ea = attn_tmp.tile([128, S], F16, tag="ea")
            sums = attn_tmp.tile([128, 1], F32, tag="sums")
            nc.scalar.activation(
                out=ea, in_=sc, func=EXP, scale=0.125, accum_out=sums
            )
            rec = attn_tmp.tile([128, 1], F32, tag="rec")
            nc.vector.reciprocal(out=rec, in_=sums)
            ean = attn_tmp.tile([128, S], F16, tag="ean")
            nc.vector.tensor_scalar(
                out=ean, in0=ea, scalar1=rec, scalar2=None, op0=MULT
            )
            nc.tensor.transpose(at_m[:, h, :], ean[:, :128], eye16)
            nc.tensor.transpose(at_t[:, h, :], ean[:, 128:S], eye16)
        atm_sb = attn_tmp.tile([128, 8, 128], F16, tag="atm_sb")
        att_sb = attn_tmp.tile([9, 8, 128], F16, tag="att_sb")
        nc.vector.tensor_copy(atm_sb, at_m)
        nc.vector.tensor_copy(att_sb, at_t)
        warm_pe(12, ps_scx, "scx", [128, 4, 128])
        for h in range(8):
            hp = 64 * (h % 2)
            hkt = h // 2
            nc.tensor.matmul(
                ctx2[hp : hp + 64, hkt, :],
                vt_sb[:, b, cq, 64 * h : 64 * h + 64],
                atm_sb[:, h, :],
                start=True, stop=False,
            )
            nc.tensor.matmul(
                ctx2[hp : hp + 64, hkt, :],
                vt_sb[0:9, b, cq + 1, 64 * h : 64 * h + 64],
                att_sb[0:9, h, :],
                start=False, stop=True,
            )
        ctxn_sb = attn_tmp.tile([128, 4, 128], F16, tag="ctxn_sb")
        nc.scalar.activation(out=ctxn_sb, in_=ctx2, func=COPY)
        op = ps_proj.tile([128, D], F32, tag="proj")
        for j in range(4):
            nc.tensor.matmul(
                op, ctxn_sb[:, j, :], w("o", j),
                start=(j == 0), stop=(j == 3),
            )
        oslot = out_stage[:, b * NQC + cq, :]
        nc.scalar.activation(out=oslot, in_=op, func=COPY)
        if b * NQC + cq < B * NQC - 1:
            nc.gpsimd.tensor_add(oslot, oslot, xq32[:, b * NQC + cq, :])
        else:
            nc.vector.tensor_add(oslot, oslot, xq32[:, b * NQC + cq, :])
        nc.sync.dma_start(outd[b, 128 * cq : 128 * (cq + 1), :], oslot)

    # pipeline: b0 LN -> b0 proj -> (b1 LN) -> b0 attn overlaps b1 proj
    emit_ln(0, 0)
    warm_pe(60, ps_at, "atm", [128, 8, 128])
    for i in range(1, NTT):
        emit_ln(0, i)
    emit_proj(0)
    for i in range(NTT):
        emit_ln(1, i)
    emit_attn(0, 0)
    emit_attn(0, 1)
    emit_proj(1)
    emit_attn(1, 0)
    emit_attn(1, 1)


def _prep_host(inputs):
    """Host-side weight folding and per-core slicing."""
    x = np.asarray(inputs["x"], np.float32)
    gamma = np.asarray(inputs["gamma"], np.float32)
    beta = np.asarray(inputs["beta"], np.float32)
    Wq = np.asarray(inputs["Wq"], np.float32).reshape(D, H * DH)
    Wk = np.asarray(inputs["Wk"], np.float32).reshape(D, H * DH)
    Wv = np.asarray(inputs["Wv"], np.float32).reshape(D, H * DH)
    Wo = np.asarray(inputs["Wo"], np.float32).reshape(H * DH, D)
    bq = np.asarray(inputs["bq"], np.float32).reshape(H * DH)
    bk = np.asarray(inputs["bk"], np.float32).reshape(H * DH)
    bv = np.asarray(inputs["bv"], np.float32).reshape(H * DH)
    bo = np.asarray(inputs["bo"], np.float32).reshape(D)

    Wq2 = gamma[:, None] * Wq
    Wk2 = gamma[:, None] * Wk
    Wv2 = gamma[:, None] * Wv
    cq = bq + beta @ Wq
    ck = bk + beta @ Wk
    cv = bv + beta @ Wv
    if np.any(cq) or np.any(ck):
        raise NotImplementedError("nonzero q/k bias not supported")
    bo_eff = bo + cv @ Wo

    wall = np.concatenate(
        [
            w.reshape(4, 128, H * DH).astype(np.float16)
            for w in (Wq2, Wk2, Wv2)
        ]
        + [Wo.reshape(4, 128, D).astype(np.float16)],
        axis=0,
    )
    wall = np.ascontiguousarray(wall)

    eye16 = np.eye(128, dtype=np.float16)

    in_maps = []
    for c in range(NCORES):
        g0 = TLOC * c - WF
        xs = np.zeros((B, TIN, D), np.float32)
        lo, hi = max(0, g0), min(T, g0 + TIN)
        xs[:, lo - g0 : hi - g0, :] = x[:, lo:hi, :]

        mask = np.full((NQC, 128, S), NEG, np.float16)
        for cqi in range(NQC):
            r = np.arange(128)[:, None]
            sl = np.arange(S)[None, :]
            gj = g0 + 128 * cqi + sl
            valid = (sl - r >= 0) & (sl - r <= WF + WB) & (gj >= 0) & (gj < T)
            mask[cqi][valid] = 0.0

        xq32 = np.ascontiguousarray(
            x[:, TLOC * c : TLOC * (c + 1), :].reshape(B, NQC, 128, D)
        )
        in_maps.append(
            {
                "xs": xs, "wall": wall,
                "maskd": mask, "eye16": eye16, "xq32": xq32,
            }
        )
    return in_maps, bo_eff


def kernel(**inputs) -> np.ndarray:
    if "nc" not in _CACHE:
        _CACHE["nc"] = _build_program()
    nc = _CACHE["nc"]
    in_maps, bo_eff = _prep_host(inputs)
    res = run_bass_kernel_spmd(nc, in_maps, list(range(NCORES)))
    out = np.empty((B, T, D), np.float32)
    for c in range(NCORES):
        out[:, TLOC * c : TLOC * (c + 1), :] = res.results[c]["out"]
    if np.any(bo_eff):
        out += bo_eff
    return out



# revision 9
# speedup vs baseline: 1.0742x; 1.0742x over previous
"""ChunkMHSA (banded local-window attention) Trainium2 kernel, v2.

Full-input contract: kernel(**inputs) takes the complete tensors from
setup_inputs() and returns the full [B, T, D] output.  Internally the
sequence dimension is sharded 8 ways (256 queries per NeuronCore) with a
front/back halo of 6/3 tokens, so each core runs the whole fused pipeline
(LayerNorm -> QKV -> banded softmax(QK^T)V -> output projection ->
residual) independently -- no collectives.

v2 changes vs baseline (94.5us):
 - fp8e4m3 DoubleRow matmuls for all four projections (4x fewer PE cycles);
   weights are scaled x64 on the host to stay in fp8 normal range and the
   1/64 compensation is folded into the psum-evacuation scales.
 - softmax batched per 2-head group: scores accumulate into [128,2,137]
   f32 psum tiles (mask pre-added via one eye-matmul per tile), ONE exp
   ACT per tile (no accum-register reads), one grouped tensor_reduce for
   the denominators, one reciprocal and one broadcast tensor_tensor for
   the normalization of all 8 heads at once.
 - ACT table thrash eliminated: all LayerNorm Sqrt ops are emitted before
   the first Exp, each table loads exactly once, off the critical path.
 - LN apply moved to ACT (Identity with per-partition scale/bias), attn
   softmax chain on DVE, residual adds on GpSimd except the last chunk
   (fused psum-scale+residual scalar_tensor_tensor on DVE).
"""

import os

os.environ.setdefault("JAX_PLATFORMS", "axon")

from contextlib import ExitStack

import numpy as np
import ml_dtypes

import concourse.bass as bass
import concourse.bacc as bacc
import concourse.tile as tile
from concourse import mybir
from concourse.bass import broadcast_tensor_aps
from concourse.bass_utils import run_bass_kernel_spmd

F32 = mybir.dt.float32
F16 = mybir.dt.float16
F8 = mybir.dt.float8e4
FP8NP = ml_dtypes.float8_e4m3fn

B, T, D = 2, 2048, 512
H, DH = 8, 64
WF, WB = 6, 3
LN_EPS = 1e-3
NCORES = 8
TLOC = T // NCORES          # 256 queries per core
TIN = WF + TLOC + WB        # 265 local tokens incl. halo
NTT = 3                     # token tiles per batch (128+128+9)
NQC = 2                     # query chunks of 128 per batch
S = 128 + WF + WB           # 137 keys per query chunk
NEG = -30000.0              # additive mask value (fp16-safe)
WS = 64.0                   # host-side fp8 weight scale
DR = mybir.MatmulPerfMode.DoubleRow
USE_DR = os.environ.get("K_DR", "1") == "1"
BATCH_MASK = os.environ.get("K_BATCHMASK", "0") == "1"

_CACHE = {}


def _build_program():
    nc = bacc.Bacc(
        "TRN2", target_bir_lowering=False, debug=False, num_devices=NCORES
    )

    xs = nc.dram_tensor("xs", [B, TIN, D], F32, kind="ExternalInput").ap()
    w8d = nc.dram_tensor("w8", [8, 128, 2, D], F8, kind="ExternalInput").ap()
    maskd = nc.dram_tensor("maskd", [NQC, 128, 2, S], F16, kind="ExternalInput").ap()
    eye16d = nc.dram_tensor("eye16", [128, 128], F16, kind="ExternalInput").ap()
    xq32d = nc.dram_tensor("xq32", [B, NQC, 128, D], F32, kind="ExternalInput").ap()
    outd = nc.dram_tensor("out", [B, TLOC, D], F32, kind="ExternalOutput").ap()

    with tile.TileContext(nc) as tc, ExitStack() as ctx:
        _emit(ctx, tc, xs, w8d, maskd, eye16d, xq32d, outd)

    nc.compile()
    return nc


def _emit(ctx, tc, xs, w8d, maskd, eye16d, xq32d, outd):
    nc = tc.nc
    EXP = mybir.ActivationFunctionType.Exp
    SQRT = mybir.ActivationFunctionType.Sqrt
    COPY = mybir.ActivationFunctionType.Copy
    IDENT = mybir.ActivationFunctionType.Identity
    SUB = mybir.AluOpType.subtract
    MULT = mybir.AluOpType.mult
    ADD = mybir.AluOpType.add

    consts = ctx.enter_context(tc.tile_pool(name="consts", bufs=1))
    persist = ctx.enter_context(tc.tile_pool(name="persist", bufs=1))
    ln_tmp = ctx.enter_context(tc.tile_pool(name="ln_tmp", bufs=3))
    attn_tmp = ctx.enter_context(tc.tile_pool(name="attn_tmp", bufs=2))
    # PSUM budget 8 banks: sc(3) proj(2) atm(1) att(1) ctx2(1)
    ps_scx = ctx.enter_context(tc.tile_pool(name="ps_scx", bufs=3, space="PSUM"))
    ps_proj = ctx.enter_context(tc.tile_pool(name="ps_proj", bufs=2, space="PSUM"))
    ps_at = ctx.enter_context(tc.tile_pool(name="ps_at", bufs=1, space="PSUM"))
    ps_att = ctx.enter_context(tc.tile_pool(name="ps_att", bufs=1, space="PSUM"))
    ps_ctx = ctx.enter_context(tc.tile_pool(name="ps_ctx", bufs=1, space="PSUM"))

    # ---- constants / weights (DMA issue spread across idle queues) ----------
    eye16 = consts.tile([128, 128], F16)
    nc.sync.dma_start(eye16, eye16d)
    mask_sb = consts.tile([128, NQC, 2, S], F16)
    nc.sync.dma_start(mask_sb, maskd.rearrange("c p u s -> p c u s"))
    xq32 = consts.tile([128, B * NQC, D], F32)
    nc.gpsimd.dma_start(xq32, xq32d.rearrange("b c p d -> p (b c) d"))
    w_sb = consts.tile([128, 8, 2, D], F8)
    nc.gpsimd.dma_start(w_sb, w8d.rearrange("m p i d -> p m i d"))
    epst = consts.tile([128, 1], F32)
    nc.vector.memset(epst, LN_EPS)
    # load the Sqrt ACT table immediately (1.3us, during the DMA prologue)
    warm = consts.tile([128, 1], F32)
    nc.scalar.activation(out=warm, in_=epst, func=SQRT, bias=epst)

    # ---- persistent tiles ---------------------------------------------------
    x_sb = persist.tile([128, 2 * NTT, D], F32)
    xtr = persist.tile([128, 4, 2 * 384], F8)    # [dpart, dchunk, b*384+tok]
    q_sb = persist.tile([128, 4, B, TLOC], F16, tag="q_sb")
    k_sb = persist.tile([128, 4, B, TIN], F16, tag="k_sb")
    vt_sb = persist.tile([128, B, NTT, D], F16, tag="vt_sb")
    out_stage = persist.tile([128, B * NQC, D], F32, tag="out_stage")

    for b in range(B):
        nc.gpsimd.memset(x_sb[:, b * NTT + 2, :], 0.0)
    # per-tile x loads so LayerNorm can start on tile 0 early
    for b in range(B):
        eng = nc.sync if b == 0 else nc.scalar
        for i in range(2):
            eng.dma_start(
                x_sb[:, b * NTT + i, :], xs[b, 128 * i : 128 * (i + 1), :]
            )
        eng.dma_start(x_sb[:9, b * NTT + 2, :], xs[b, 256:TIN, :])

    def emit_ln(b, i):
        xt = x_sb[:, b * NTT + i, :]
        st = ln_tmp.tile([128, 6], F32, tag="st")
        mv = ln_tmp.tile([128, 2], F32, tag="mv")
        nc.vector.bn_stats(out=st, in_=xt)
        nc.vector.bn_aggr(out=mv, in_=st)
        sd = ln_tmp.tile([128, 1], F32, tag="sd")
        nc.scalar.activation(out=sd, in_=mv[:, 1:2], func=SQRT, bias=epst)
        rstd = ln_tmp.tile([128, 1], F32, tag="rstd")
        nc.vector.reciprocal(out=rstd, in_=sd)
        # bias = -mu * rstd so ACT can apply LN as Identity(x*rstd + bias)
        nmr = ln_tmp.tile([128, 1], F32, tag="nmr")
        nc.vector.tensor_scalar(
            out=nmr, in0=mv[:, 0:1], scalar1=rstd, scalar2=-1.0,
            op0=MULT, op1=MULT,
        )
        xr = ln_tmp.tile([128, D], F16, tag="xr")
        nc.scalar.activation(out=xr, in_=xt, func=IDENT, bias=nmr, scale=rstd)
        pt = ps_scx.tile([128, 4, 128], F16, tag="sc")
        for j in range(4):
            nc.tensor.transpose(pt[:, j, :], xr[:, 128 * j : 128 * j + 128], eye16)
        dst = xtr[:, :, 384 * b + 128 * i : 384 * b + 128 * (i + 1)]
        if i % 2 == 0:
            nc.vector.tensor_copy(dst, pt)
        else:
            nc.scalar.activation(out=dst, in_=pt, func=COPY)

    def w(widx, t):
        # [128, 2, D] fp8 DoubleRow pair t of matrix widx (0=q,1=k,2=v,3=o)
        return w_sb[:, 2 * widx + t, :, :]

    def emit_proj_qk(b):
        # q: queries only (N=256)
        for hkt in range(4):
            pp = ps_proj.tile([128, D], F32, tag="proj")
            for t in range(2):
                if USE_DR:
                    nc.tensor.matmul(
                        pp[:, 0:TLOC],
                        w(0, t)[:, :, 128 * hkt : 128 * (hkt + 1)],
                        xtr[:, 2 * t : 2 * t + 2, 384 * b + WF : 384 * b + WF + TLOC],
                        start=(t == 0), stop=(t == 1), perf_mode=DR,
                    )
                else:
                    for i2 in range(2):
                        nc.tensor.matmul(
                            pp[:, 0:TLOC],
                            w(0, t)[:, i2, 128 * hkt : 128 * (hkt + 1)],
                            xtr[:, 2 * t + i2, 384 * b + WF : 384 * b + WF + TLOC],
                            start=(t == 0 and i2 == 0), stop=(t == 1 and i2 == 1),
                        )
            if hkt % 2 == 0:
                nc.vector.tensor_scalar(
                    out=q_sb[:, hkt, b, :], in0=pp[:, 0:TLOC],
                    scalar1=1.0 / WS, scalar2=None, op0=MULT,
                )
            else:
                nc.scalar.activation(
                    out=q_sb[:, hkt, b, :], in_=pp[:, 0:TLOC],
                    func=COPY, scale=1.0 / WS,
                )
        # k incl. halo (N=265)
        for hkt in range(4):
            pp = ps_proj.tile([128, D], F32, tag="proj")
            for t in range(2):
                if USE_DR:
                    nc.tensor.matmul(
                        pp[:, 0:TIN],
                        w(1, t)[:, :, 128 * hkt : 128 * (hkt + 1)],
                        xtr[:, 2 * t : 2 * t + 2, 384 * b : 384 * b + TIN],
                        start=(t == 0), stop=(t == 1), perf_mode=DR,
                    )
                else:
                    for i2 in range(2):
                        nc.tensor.matmul(
                            pp[:, 0:TIN],
                            w(1, t)[:, i2, 128 * hkt : 128 * (hkt + 1)],
                            xtr[:, 2 * t + i2, 384 * b : 384 * b + TIN],
                            start=(t == 0 and i2 == 0), stop=(t == 1 and i2 == 1),
                        )
            nc.scalar.activation(
                out=k_sb[:, hkt, b, :], in_=pp[:, 0:TIN],
                func=COPY, scale=1.0 / WS,
            )

    def emit_proj_v(b):
        # vT per token tile (N=512)
        for i in range(NTT):
            pp = ps_proj.tile([128, D], F32, tag="proj")
            for t in range(2):
                if USE_DR:
                    nc.tensor.matmul(
                        pp,
                        xtr[:, 2 * t : 2 * t + 2, 384 * b + 128 * i : 384 * b + 128 * (i + 1)],
                        w(2, t),
                        start=(t == 0), stop=(t == 1), perf_mode=DR,
                    )
                else:
                    for i2 in range(2):
                        nc.tensor.matmul(
                            pp,
                            xtr[:, 2 * t + i2, 384 * b + 128 * i : 384 * b + 128 * (i + 1)],
                            w(2, t)[:, i2, :],
                            start=(t == 0 and i2 == 0), stop=(t == 1 and i2 == 1),
                        )
            if i % 2 == 0:
                nc.scalar.activation(
                    out=vt_sb[:, b, i, :], in_=pp, func=COPY, scale=1.0 / WS
                )
            else:
                nc.vector.tensor_scalar(
                    out=vt_sb[:, b, i, :], in0=pp,
                    scalar1=1.0 / WS, scalar2=None, op0=MULT,
                )

    def emit_attn_a(b, cq, ea):
        """Scores + exp for all 8 heads of one query chunk."""
        q0 = 128 * cq
        s0 = 128 * cq
        for j in range(4):
            sc = ps_scx.tile([128, 2, S], F32, tag="sc")
            if BATCH_MASK:
                nc.tensor.matmul(
                    sc, eye16, mask_sb[:, cq, :, :],
                    start=True, stop=False, skip_group_check=True,
                )
            for u in range(2):
                hp = 64 * u
                if not BATCH_MASK:
                    nc.tensor.matmul(
                        sc[:, u, :], eye16, mask_sb[:, cq, u, :],
                        start=True, stop=False,
                    )
                nc.tensor.matmul(
                    sc[:, u, :],
                    q_sb[hp : hp + 64, j, b, q0 : q0 + 128],
                    k_sb[hp : hp + 64, j, b, s0 : s0 + S],
                    start=False, stop=True, skip_group_check=BATCH_MASK,
                )
            nc.scalar.activation(
                out=ea[:, 2 * j : 2 * j + 2, :], in_=sc, func=EXP, scale=0.125
            )

    def emit_attn_b(b, cq, ea, last):
        """Softmax normalize + ctx + out-proj + residual + store."""
        sums = attn_tmp.tile([128, 8], F32, tag="sums")
        nc.vector.reduce_sum(out=sums, in_=ea, axis=mybir.AxisListType.X)
        rec = attn_tmp.tile([128, 8, 1], F32, tag="rec")
        nc.vector.reciprocal(out=rec[:, :, 0], in_=sums)
        ean = attn_tmp.tile([128, 8, S], F16, tag="ean")
        for h in range(8):
            nc.vector.tensor_scalar(
                out=ean[:, h, :], in0=ea[:, h, :],
                scalar1=rec[:, h, :], scalar2=None, op0=MULT,
            )

        at_m = ps_at.tile([128, 8, 128], F16, tag="atm")
        at_t = ps_att.tile([9, 8, 128], F16, tag="att")
        for h in range(8):
            nc.tensor.transpose(at_m[:, h, :], ean[:, h, 0:128], eye16)
        for h in range(8):
            nc.tensor.transpose(at_t[:, h, :], ean[:, h, 128:S], eye16)
        atm_sb = attn_tmp.tile([128, 8, 128], F16, tag="atm_sb")
        att_sb = attn_tmp.tile([9, 8, 128], F16, tag="att_sb")
        nc.vector.tensor_copy(atm_sb, at_m)
        nc.vector.tensor_copy(att_sb, at_t)

        ctx2 = ps_ctx.tile([128, 4, 128], F32, tag="ctx2")
        for h in range(8):
            hp = 64 * (h % 2)
            hkt = h // 2
            nc.tensor.matmul(
                ctx2[hp : hp + 64, hkt, :],
                vt_sb[:, b, cq, 64 * h : 64 * h + 64],
                atm_sb[:, h, :],
                start=True, stop=False,
            )
            nc.tensor.matmul(
                ctx2[hp : hp + 64, hkt, :],
                vt_sb[0:9, b, cq + 1, 64 * h : 64 * h + 64],
                att_sb[0:9, h, :],
                start=False, stop=True,
            )
        ctxn = attn_tmp.tile([128, 4, 128], F8, tag="ctxn")
        nc.scalar.activation(out=ctxn, in_=ctx2, func=COPY)

        op = ps_proj.tile([128, D], F32, tag="proj")
        for t in range(2):
            if USE_DR:
                nc.tensor.matmul(
                    op, ctxn[:, 2 * t : 2 * t + 2, :], w(3, t),
                    start=(t == 0), stop=(t == 1), perf_mode=DR,
                )
            else:
                for i2 in range(2):
                    nc.tensor.matmul(
                        op, ctxn[:, 2 * t + i2, :], w(3, t)[:, i2, :],
                        start=(t == 0 and i2 == 0), stop=(t == 1 and i2 == 1),
                    )
        idx = b * NQC + cq
        oslot = out_stage[:, idx, :]
        if last:
            nc.vector.scalar_tensor_tensor(
                out=oslot, in0=op, scalar=1.0 / WS, in1=xq32[:, idx, :],
                op0=MULT, op1=ADD,
            )
        else:
            nc.scalar.activation(out=oslot, in_=op, func=COPY, scale=1.0 / WS)
            nc.gpsimd.tensor_add(oslot, oslot, xq32[:, idx, :])
        nc.sync.dma_start(outd[b, 128 * cq : 128 * (cq + 1), :], oslot)

    # ---- schedule -----------------------------------------------------------
    for i in range(NTT):
        emit_ln(0, i)
    emit_proj_qk(0)
    for i in range(NTT):
        emit_ln(1, i)
    # load the Exp ACT table now: after the last Sqrt, before the first exp
    warm2 = consts.tile([128, 1], F32)
    nc.scalar.activation(out=warm2, in_=epst, func=EXP)
    emit_proj_v(0)

    ea_pool = {}
    for key in [(0, 0), (0, 1), (1, 0), (1, 1)]:
        ea_pool[key] = attn_tmp.tile([128, 8, S], F16, tag="ea", name="ea")

    emit_attn_a(0, 0, ea_pool[(0, 0)])
    emit_proj_qk(1)
    emit_attn_b(0, 0, ea_pool[(0, 0)], last=False)
    emit_attn_a(0, 1, ea_pool[(0, 1)])
    emit_proj_v(1)
    emit_attn_b(0, 1, ea_pool[(0, 1)], last=False)
    emit_attn_a(1, 0, ea_pool[(1, 0)])
    emit_attn_b(1, 0, ea_pool[(1, 0)], last=False)
    emit_attn_a(1, 1, ea_pool[(1, 1)])
    emit_attn_b(1, 1, ea_pool[(1, 1)], last=True)


def _dr_pack(W):
    """[D, M] -> [2, 128, 2, M] DoubleRow k-tile pairs."""
    W4 = W.reshape(4, 128, -1)
    return np.stack(
        [np.stack([W4[2 * t], W4[2 * t + 1]], axis=1) for t in range(2)]
    )


def _prep_host(inputs):
    """Host-side weight folding and per-core slicing."""
    x = np.asarray(inputs["x"], np.float32)
    gamma = np.asarray(inputs["gamma"], np.float32)
    beta = np.asarray(inputs["beta"], np.float32)
    Wq = np.asarray(inputs["Wq"], np.float32).reshape(D, H * DH)
    Wk = np.asarray(inputs["Wk"], np.float32).reshape(D, H * DH)
    Wv = np.asarray(inputs["Wv"], np.float32).reshape(D, H * DH)
    Wo = np.asarray(inputs["Wo"], np.float32).reshape(H * DH, D)
    bq = np.asarray(inputs["bq"], np.float32).reshape(H * DH)
    bk = np.asarray(inputs["bk"], np.float32).reshape(H * DH)
    bv = np.asarray(inputs["bv"], np.float32).reshape(H * DH)
    bo = np.asarray(inputs["bo"], np.float32).reshape(D)

    Wq2 = gamma[:, None] * Wq
    Wk2 = gamma[:, None] * Wk
    Wv2 = gamma[:, None] * Wv
    cq = bq + beta @ Wq
    ck = bk + beta @ Wk
    cv = bv + beta @ Wv
    if np.any(cq) or np.any(ck):
        raise NotImplementedError("nonzero q/k bias not supported")
    bo_eff = bo + cv @ Wo

    w8 = np.concatenate(
        [_dr_pack(WS * m) for m in (Wq2, Wk2, Wv2, Wo)], axis=0
    ).astype(FP8NP)
    w8 = np.ascontiguousarray(w8)

    eye16 = np.eye(128, dtype=np.float16)

    in_maps = []
    for c in range(NCORES):
        g0 = TLOC * c - WF
        xs = np.zeros((B, TIN, D), np.float32)
        lo, hi = max(0, g0), min(T, g0 + TIN)
        xs[:, lo - g0 : hi - g0, :] = x[:, lo:hi, :]

        mask = np.full((NQC, 128, S), NEG, np.float16)
        for cqi in range(NQC):
            r = np.arange(128)[:, None]
            sl = np.arange(S)[None, :]
            gj = g0 + 128 * cqi + sl
            valid = (sl - r >= 0) & (sl - r <= WF + WB) & (gj >= 0) & (gj < T)
            mask[cqi][valid] = 0.0
        mask2 = np.ascontiguousarray(
            np.repeat(mask[:, :, None, :], 2, axis=2)
        )

        xq32 = np.ascontiguousarray(
            x[:, TLOC * c : TLOC * (c + 1), :].reshape(B, NQC, 128, D)
        )
        in_maps.append(
            {
                "xs": xs, "w8": w8, "maskd": mask2,
                "eye16": eye16, "xq32": xq32,
            }
        )
    return in_maps, bo_eff


def kernel(**inputs) -> np.ndarray:
    if "nc" not in _CACHE:
        _CACHE["nc"] = _build_program()
    nc = _CACHE["nc"]
    in_maps, bo_eff = _prep_host(inputs)
    res = run_bass_kernel_spmd(nc, in_maps, list(range(NCORES)))
    out = np.empty((B, T, D), np.float32)
    for c in range(NCORES):
        out[:, TLOC * c : TLOC * (c + 1), :] = res.results[c]["out"]
    if np.any(bo_eff):
        out += bo_eff
    return out


# revision 10
# speedup vs baseline: 1.0943x; 1.0187x over previous
"""ChunkMHSA (banded local-window attention) Trainium2 kernel, v2.

Full-input contract: kernel(**inputs) takes the complete tensors from
setup_inputs() and returns the full [B, T, D] output.  Internally the
sequence dimension is sharded 8 ways (256 queries per NeuronCore) with a
front/back halo of 6/3 tokens, so each core runs the whole fused pipeline
(LayerNorm -> QKV -> banded softmax(QK^T)V -> output projection ->
residual) independently -- no collectives.

v2 changes vs baseline (94.5us):
 - fp8e4m3 DoubleRow matmuls for all four projections (4x fewer PE cycles);
   weights are scaled x64 on the host to stay in fp8 normal range and the
   1/64 compensation is folded into the psum-evacuation scales.
 - softmax batched per 2-head group: scores accumulate into [128,2,137]
   f32 psum tiles (mask pre-added via one eye-matmul per tile), ONE exp
   ACT per tile (no accum-register reads), one grouped tensor_reduce for
   the denominators, one reciprocal and one broadcast tensor_tensor for
   the normalization of all 8 heads at once.
 - ACT table thrash eliminated: all LayerNorm Sqrt ops are emitted before
   the first Exp, each table loads exactly once, off the critical path.
 - LN apply moved to ACT (Identity with per-partition scale/bias), attn
   softmax chain on DVE, residual adds on GpSimd except the last chunk
   (fused psum-scale+residual scalar_tensor_tensor on DVE).
"""

import os

os.environ.setdefault("JAX_PLATFORMS", "axon")

from contextlib import ExitStack

import numpy as np
import ml_dtypes

import concourse.bass as bass
import concourse.bacc as bacc
import concourse.tile as tile
from concourse import mybir
from concourse.bass import broadcast_tensor_aps
from concourse.bass_utils import run_bass_kernel_spmd

F32 = mybir.dt.float32
F16 = mybir.dt.float16
F8 = mybir.dt.float8e4
FP8NP = ml_dtypes.float8_e4m3fn

B, T, D = 2, 2048, 512
H, DH = 8, 64
WF, WB = 6, 3
LN_EPS = 1e-3
NCORES = 8
TLOC = T // NCORES          # 256 queries per core
TIN = WF + TLOC + WB        # 265 local tokens incl. halo
NTT = 3                     # token tiles per batch (128+128+9)
NQC = 2                     # query chunks of 128 per batch
S = 128 + WF + WB           # 137 keys per query chunk
NEG = -30000.0              # additive mask value (fp16-safe)
WS = 64.0                   # host-side fp8 weight scale
DR = mybir.MatmulPerfMode.DoubleRow
USE_DR = os.environ.get("K_DR", "1") == "1"
BATCH_MASK = os.environ.get("K_BATCHMASK", "0") == "1"

_CACHE = {}


def _build_program():
    nc = bacc.Bacc(
        "TRN2", target_bir_lowering=False, debug=False, num_devices=NCORES
    )

    xs = nc.dram_tensor("xs", [B, TIN, D], F32, kind="ExternalInput").ap()
    w8d = nc.dram_tensor("w8", [128, 8, 2, D], F8, kind="ExternalInput").ap()
    maskd = nc.dram_tensor("maskd", [128, NQC, 2, S], F16, kind="ExternalInput").ap()
    eye16d = nc.dram_tensor("eye16", [128, 128], F16, kind="ExternalInput").ap()
    xq32d = nc.dram_tensor("xq32", [128, B * NQC, D], F32, kind="ExternalInput").ap()
    outd = nc.dram_tensor("out", [B, TLOC, D], F32, kind="ExternalOutput").ap()

    with tile.TileContext(nc) as tc, ExitStack() as ctx:
        _emit(ctx, tc, xs, w8d, maskd, eye16d, xq32d, outd)

    nc.compile()
    return nc


def _emit(ctx, tc, xs, w8d, maskd, eye16d, xq32d, outd):
    nc = tc.nc
    EXP = mybir.ActivationFunctionType.Exp
    SQRT = mybir.ActivationFunctionType.Sqrt
    COPY = mybir.ActivationFunctionType.Copy
    IDENT = mybir.ActivationFunctionType.Identity
    SUB = mybir.AluOpType.subtract
    MULT = mybir.AluOpType.mult
    ADD = mybir.AluOpType.add

    consts = ctx.enter_context(tc.tile_pool(name="consts", bufs=1))
    persist = ctx.enter_context(tc.tile_pool(name="persist", bufs=1))
    ln_tmp = ctx.enter_context(tc.tile_pool(name="ln_tmp", bufs=3))
    attn_tmp = ctx.enter_context(tc.tile_pool(name="attn_tmp", bufs=2))
    # PSUM budget 8 banks: sc(3) proj(2) atm(1) att(1) ctx2(1)
    ps_scx = ctx.enter_context(tc.tile_pool(name="ps_scx", bufs=3, space="PSUM"))
    ps_proj = ctx.enter_context(tc.tile_pool(name="ps_proj", bufs=2, space="PSUM"))
    ps_at = ctx.enter_context(tc.tile_pool(name="ps_at", bufs=1, space="PSUM"))
    ps_att = ctx.enter_context(tc.tile_pool(name="ps_att", bufs=1, space="PSUM"))
    ps_ctx = ctx.enter_context(tc.tile_pool(name="ps_ctx", bufs=1, space="PSUM"))

    # ---- constants / weights (DMA issue spread across idle queues) ----------
    eye16 = consts.tile([128, 128], F16)
    nc.sync.dma_start(eye16, eye16d)
    mask_sb = consts.tile([128, NQC, 2, S], F16)
    nc.sync.dma_start(mask_sb, maskd)
    w_sb = consts.tile([128, 8, 2, D], F8)
    nc.gpsimd.dma_start(w_sb, w8d)
    xq32 = consts.tile([128, B * NQC, D], F32)
    nc.gpsimd.dma_start(xq32, xq32d)
    epst = consts.tile([128, 1], F32)
    nc.vector.memset(epst, LN_EPS)
    # load the Sqrt ACT table immediately (1.3us, during the DMA prologue)
    warm = consts.tile([128, 1], F32)
    nc.scalar.activation(out=warm, in_=epst, func=SQRT, bias=epst)

    # ---- persistent tiles ---------------------------------------------------
    x_sb = persist.tile([128, 2 * NTT, D], F32)
    xtr = persist.tile([128, 4, 2 * 384], F8)    # [dpart, dchunk, b*384+tok]
    q_sb = persist.tile([128, 4, B, TLOC], F16, tag="q_sb")
    k_sb = persist.tile([128, 4, B, TIN], F16, tag="k_sb")
    vt_sb = persist.tile([128, B, NTT, D], F16, tag="vt_sb")
    out_stage = persist.tile([128, B * NQC, D], F32, tag="out_stage")

    for b in range(B):
        nc.gpsimd.memset(x_sb[:, b * NTT + 2, :], 0.0)
    # per-tile x loads so LayerNorm can start on tile 0 early
    for b in range(B):
        eng = nc.sync if b == 0 else nc.scalar
        for i in range(2):
            eng.dma_start(
                x_sb[:, b * NTT + i, :], xs[b, 128 * i : 128 * (i + 1), :]
            )
        eng.dma_start(x_sb[:9, b * NTT + 2, :], xs[b, 256:TIN, :])

    mvall = persist.tile([128, 2 * NTT, 2], F32, tag="mvall")
    rstd_all = persist.tile([128, 2 * NTT], F32, tag="rstd_all")
    nmr_all = persist.tile([128, 2 * NTT], F32, tag="nmr_all")

    def emit_ln_stats(b, i):
        xt = x_sb[:, b * NTT + i, :]
        st = ln_tmp.tile([128, 6], F32, tag="st")
        nc.vector.bn_stats(out=st, in_=xt)
        nc.vector.bn_aggr(out=mvall[:, b * NTT + i, :], in_=st)

    def emit_ln_rstd():
        # one Sqrt ACT for all 6 tiles: the only Sqrt-table op in the program
        sd_all = ln_tmp.tile([128, 2 * NTT], F32, tag="sd_all")
        nc.scalar.activation(
            out=sd_all, in_=mvall[:, :, 1], func=SQRT, bias=epst
        )
        nc.vector.reciprocal(out=rstd_all, in_=sd_all)
        # nmr = -mean * rstd (per-tile per-partition LN bias)
        nc.vector.scalar_tensor_tensor(
            out=nmr_all, in0=mvall[:, :, 0], scalar=-1.0, in1=rstd_all,
            op0=MULT, op1=MULT,
        )

    def emit_ln_apply(b, i):
        xt = x_sb[:, b * NTT + i, :]
        idx = b * NTT + i
        xr = ln_tmp.tile([128, D], F16, tag="xr")
        nc.scalar.activation(
            out=xr, in_=xt, func=IDENT,
            bias=nmr_all[:, idx : idx + 1], scale=rstd_all[:, idx : idx + 1],
        )
        pt = ps_scx.tile([128, 4, 128], F16, tag="sc")
        for j in range(4):
            nc.tensor.transpose(pt[:, j, :], xr[:, 128 * j : 128 * j + 128], eye16)
        dst = xtr[:, :, 384 * b + 128 * i : 384 * b + 128 * (i + 1)]
        if i % 2 == 0:
            nc.vector.tensor_copy(dst, pt)
        else:
            nc.scalar.activation(out=dst, in_=pt, func=COPY)

    def w(widx, t):
        # [128, 2, D] fp8 DoubleRow pair t of matrix widx (0=q,1=k,2=v,3=o)
        return w_sb[:, 2 * widx + t, :, :]

    def emit_proj_qk(b):
        # q: queries only (N=256)
        for hkt in range(4):
            pp = ps_proj.tile([128, D], F32, tag="proj")
            for t in range(2):
                if USE_DR:
                    nc.tensor.matmul(
                        pp[:, 0:TLOC],
                        w(0, t)[:, :, 128 * hkt : 128 * (hkt + 1)],
                        xtr[:, 2 * t : 2 * t + 2, 384 * b + WF : 384 * b + WF + TLOC],
                        start=(t == 0), stop=(t == 1), perf_mode=DR,
                    )
                else:
                    for i2 in range(2):
                        nc.tensor.matmul(
                            pp[:, 0:TLOC],
                            w(0, t)[:, i2, 128 * hkt : 128 * (hkt + 1)],
                            xtr[:, 2 * t + i2, 384 * b + WF : 384 * b + WF + TLOC],
                            start=(t == 0 and i2 == 0), stop=(t == 1 and i2 == 1),
                        )
            if hkt % 2 == 0:
                nc.vector.tensor_scalar(
                    out=q_sb[:, hkt, b, :], in0=pp[:, 0:TLOC],
                    scalar1=1.0 / WS, scalar2=None, op0=MULT,
                )
            else:
                nc.scalar.activation(
                    out=q_sb[:, hkt, b, :], in_=pp[:, 0:TLOC],
                    func=COPY, scale=1.0 / WS,
                )
        # k incl. halo (N=265)
        for hkt in range(4):
            pp = ps_proj.tile([128, D], F32, tag="proj")
            for t in range(2):
                if USE_DR:
                    nc.tensor.matmul(
                        pp[:, 0:TIN],
                        w(1, t)[:, :, 128 * hkt : 128 * (hkt + 1)],
                        xtr[:, 2 * t : 2 * t + 2, 384 * b : 384 * b + TIN],
                        start=(t == 0), stop=(t == 1), perf_mode=DR,
                    )
                else:
                    for i2 in range(2):
                        nc.tensor.matmul(
                            pp[:, 0:TIN],
                            w(1, t)[:, i2, 128 * hkt : 128 * (hkt + 1)],
                            xtr[:, 2 * t + i2, 384 * b : 384 * b + TIN],
                            start=(t == 0 and i2 == 0), stop=(t == 1 and i2 == 1),
                        )
            nc.scalar.activation(
                out=k_sb[:, hkt, b, :], in_=pp[:, 0:TIN],
                func=COPY, scale=1.0 / WS,
            )

    def emit_proj_v(b):
        # vT per token tile (N=512)
        for i in range(NTT):
            pp = ps_proj.tile([128, D], F32, tag="proj")
            for t in range(2):
                if USE_DR:
                    nc.tensor.matmul(
                        pp,
                        xtr[:, 2 * t : 2 * t + 2, 384 * b + 128 * i : 384 * b + 128 * (i + 1)],
                        w(2, t),
                        start=(t == 0), stop=(t == 1), perf_mode=DR,
                    )
                else:
                    for i2 in range(2):
                        nc.tensor.matmul(
                            pp,
                            xtr[:, 2 * t + i2, 384 * b + 128 * i : 384 * b + 128 * (i + 1)],
                            w(2, t)[:, i2, :],
                            start=(t == 0 and i2 == 0), stop=(t == 1 and i2 == 1),
                        )
            if i % 2 == 0:
                nc.scalar.activation(
                    out=vt_sb[:, b, i, :], in_=pp, func=COPY, scale=1.0 / WS
                )
            else:
                nc.vector.tensor_scalar(
                    out=vt_sb[:, b, i, :], in0=pp,
                    scalar1=1.0 / WS, scalar2=None, op0=MULT,
                )

    def emit_attn_a(b, cq, ea):
        """Scores + exp for all 8 heads of one query chunk."""
        q0 = 128 * cq
        s0 = 128 * cq
        for j in range(4):
            sc = ps_scx.tile([128, 2, S], F32, tag="sc")
            if BATCH_MASK:
                nc.tensor.matmul(
                    sc, eye16, mask_sb[:, cq, :, :],
                    start=True, stop=False, skip_group_check=True,
                )
            for u in range(2):
                hp = 64 * u
                if not BATCH_MASK:
                    nc.tensor.matmul(
                        sc[:, u, :], eye16, mask_sb[:, cq, u, :],
                        start=True, stop=False,
                    )
                nc.tensor.matmul(
                    sc[:, u, :],
                    q_sb[hp : hp + 64, j, b, q0 : q0 + 128],
                    k_sb[hp : hp + 64, j, b, s0 : s0 + S],
                    start=False, stop=True, skip_group_check=BATCH_MASK,
                )
            nc.scalar.activation(
                out=ea[:, 2 * j : 2 * j + 2, :], in_=sc, func=EXP, scale=0.125
            )

    def emit_attn_b(b, cq, ea, last):
        """Softmax normalize + ctx + out-proj + residual + store."""
        sums = attn_tmp.tile([128, 8], F32, tag="sums")
        nc.vector.reduce_sum(out=sums, in_=ea, axis=mybir.AxisListType.X)
        rec = attn_tmp.tile([128, 8, 1], F32, tag="rec")
        nc.vector.reciprocal(out=rec[:, :, 0], in_=sums)
        ean = attn_tmp.tile([128, 8, S], F16, tag="ean")
        for h in range(8):
            nc.vector.tensor_scalar(
                out=ean[:, h, :], in0=ea[:, h, :],
                scalar1=rec[:, h, :], scalar2=None, op0=MULT,
            )

        at_m = ps_at.tile([128, 8, 128], F16, tag="atm")
        at_t = ps_att.tile([9, 8, 128], F16, tag="att")
        for h in range(8):
            nc.tensor.transpose(at_m[:, h, :], ean[:, h, 0:128], eye16)
        for h in range(8):
            nc.tensor.transpose(at_t[:, h, :], ean[:, h, 128:S], eye16)
        atm_sb = attn_tmp.tile([128, 8, 128], F16, tag="atm_sb")
        att_sb = attn_tmp.tile([9, 8, 128], F16, tag="att_sb")
        nc.vector.tensor_copy(atm_sb, at_m)
        nc.vector.tensor_copy(att_sb, at_t)

        ctx2 = ps_ctx.tile([128, 4, 128], F32, tag="ctx2")
        for h in range(8):
            hp = 64 * (h % 2)
            hkt = h // 2
            nc.tensor.matmul(
                ctx2[hp : hp + 64, hkt, :],
                vt_sb[:, b, cq, 64 * h : 64 * h + 64],
                atm_sb[:, h, :],
                start=True, stop=False,
            )
            nc.tensor.matmul(
                ctx2[hp : hp + 64, hkt, :],
                vt_sb[0:9, b, cq + 1, 64 * h : 64 * h + 64],
                att_sb[0:9, h, :],
                start=False, stop=True,
            )
        ctxn = attn_tmp.tile([128, 4, 128], F8, tag="ctxn")
        nc.scalar.activation(out=ctxn, in_=ctx2, func=COPY)

        op = ps_proj.tile([128, D], F32, tag="proj")
        for t in range(2):
            if USE_DR:
                nc.tensor.matmul(
                    op, ctxn[:, 2 * t : 2 * t + 2, :], w(3, t),
                    start=(t == 0), stop=(t == 1), perf_mode=DR,
                )
            else:
                for i2 in range(2):
                    nc.tensor.matmul(
                        op, ctxn[:, 2 * t + i2, :], w(3, t)[:, i2, :],
                        start=(t == 0 and i2 == 0), stop=(t == 1 and i2 == 1),
                    )
        idx = b * NQC + cq
        oslot = out_stage[:, idx, :]
        if last:
            nc.vector.scalar_tensor_tensor(
                out=oslot, in0=op, scalar=1.0 / WS, in1=xq32[:, idx, :],
                op0=MULT, op1=ADD,
            )
        else:
            nc.scalar.activation(out=oslot, in_=op, func=COPY, scale=1.0 / WS)
            nc.gpsimd.tensor_add(oslot, oslot, xq32[:, idx, :])
        nc.sync.dma_start(outd[b, 128 * cq : 128 * (cq + 1), :], oslot)

    # ---- schedule -----------------------------------------------------------
    for b in range(B):
        for i in range(NTT):
            emit_ln_stats(b, i)
    emit_ln_rstd()
    # Exp table load right after the single Sqrt: exactly two table loads
    warm2 = consts.tile([128, 1], F32)
    nc.scalar.activation(out=warm2, in_=epst, func=EXP)
    for b in range(B):
        for i in range(NTT):
            emit_ln_apply(b, i)
    emit_proj_qk(0)
    emit_proj_v(0)

    ea_pool = {}
    for key in [(0, 0), (0, 1), (1, 0), (1, 1)]:
        ea_pool[key] = attn_tmp.tile([128, 8, S], F16, tag="ea", name="ea")

    emit_attn_a(0, 0, ea_pool[(0, 0)])
    emit_proj_qk(1)
    emit_attn_b(0, 0, ea_pool[(0, 0)], last=False)
    emit_attn_a(0, 1, ea_pool[(0, 1)])
    emit_proj_v(1)
    emit_attn_b(0, 1, ea_pool[(0, 1)], last=False)
    emit_attn_a(1, 0, ea_pool[(1, 0)])
    emit_attn_b(1, 0, ea_pool[(1, 0)], last=False)
    emit_attn_a(1, 1, ea_pool[(1, 1)])
    emit_attn_b(1, 1, ea_pool[(1, 1)], last=True)


def _dr_pack(W):
    """[D, M] -> [2, 128, 2, M] DoubleRow k-tile pairs."""
    W4 = W.reshape(4, 128, -1)
    return np.stack(
        [np.stack([W4[2 * t], W4[2 * t + 1]], axis=1) for t in range(2)]
    )


def _prep_host(inputs):
    """Host-side weight folding and per-core slicing."""
    x = np.asarray(inputs["x"], np.float32)
    gamma = np.asarray(inputs["gamma"], np.float32)
    beta = np.asarray(inputs["beta"], np.float32)
    Wq = np.asarray(inputs["Wq"], np.float32).reshape(D, H * DH)
    Wk = np.asarray(inputs["Wk"], np.float32).reshape(D, H * DH)
    Wv = np.asarray(inputs["Wv"], np.float32).reshape(D, H * DH)
    Wo = np.asarray(inputs["Wo"], np.float32).reshape(H * DH, D)
    bq = np.asarray(inputs["bq"], np.float32).reshape(H * DH)
    bk = np.asarray(inputs["bk"], np.float32).reshape(H * DH)
    bv = np.asarray(inputs["bv"], np.float32).reshape(H * DH)
    bo = np.asarray(inputs["bo"], np.float32).reshape(D)

    Wq2 = gamma[:, None] * Wq
    Wk2 = gamma[:, None] * Wk
    Wv2 = gamma[:, None] * Wv
    cq = bq + beta @ Wq
    ck = bk + beta @ Wk
    cv = bv + beta @ Wv
    if np.any(cq) or np.any(ck):
        raise NotImplementedError("nonzero q/k bias not supported")
    bo_eff = bo + cv @ Wo

    w8 = np.concatenate(
        [_dr_pack(WS * m) for m in (Wq2, Wk2, Wv2, Wo)], axis=0
    ).astype(FP8NP)
    # device layout [p, m, i, d] so the weight DMA is contiguous per partition
    w8 = np.ascontiguousarray(w8.transpose(1, 0, 2, 3))

    eye16 = np.eye(128, dtype=np.float16)

    in_maps = []
    for c in range(NCORES):
        g0 = TLOC * c - WF
        xs = np.zeros((B, TIN, D), np.float32)
        lo, hi = max(0, g0), min(T, g0 + TIN)
        xs[:, lo - g0 : hi - g0, :] = x[:, lo:hi, :]

        mask = np.full((NQC, 128, S), NEG, np.float16)
        for cqi in range(NQC):
            r = np.arange(128)[:, None]
            sl = np.arange(S)[None, :]
            gj = g0 + 128 * cqi + sl
            valid = (sl - r >= 0) & (sl - r <= WF + WB) & (gj >= 0) & (gj < T)
            mask[cqi][valid] = 0.0
        mask2 = np.ascontiguousarray(
            np.repeat(mask[:, :, None, :], 2, axis=2).transpose(1, 0, 2, 3)
        )

        xq32 = np.ascontiguousarray(
            x[:, TLOC * c : TLOC * (c + 1), :]
            .reshape(B, NQC, 128, D)
            .transpose(2, 0, 1, 3)
            .reshape(128, B * NQC, D)
        )
        in_maps.append(
            {
                "xs": xs, "w8": w8, "maskd": mask2,
                "eye16": eye16, "xq32": xq32,
            }
        )
    return in_maps, bo_eff


def kernel(**inputs) -> np.ndarray:
    if "nc" not in _CACHE:
        _CACHE["nc"] = _build_program()
    nc = _CACHE["nc"]
    in_maps, bo_eff = _prep_host(inputs)
    res = run_bass_kernel_spmd(nc, in_maps, list(range(NCORES)))
    out = np.empty((B, T, D), np.float32)
    for c in range(NCORES):
        out[:, TLOC * c : TLOC * (c + 1), :] = res.results[c]["out"]
    if np.any(bo_eff):
        out += bo_eff
    return out


# revision 12
# speedup vs baseline: 1.1928x; 1.0900x over previous
"""ChunkMHSA (banded local-window attention) Trainium2 kernel, v2.

Full-input contract: kernel(**inputs) takes the complete tensors from
setup_inputs() and returns the full [B, T, D] output.  Internally the
sequence dimension is sharded 8 ways (256 queries per NeuronCore) with a
front/back halo of 6/3 tokens, so each core runs the whole fused pipeline
(LayerNorm -> QKV -> banded softmax(QK^T)V -> output projection ->
residual) independently -- no collectives.

v2 changes vs baseline (94.5us):
 - fp8e4m3 DoubleRow matmuls for all four projections (4x fewer PE cycles);
   weights are scaled x64 on the host to stay in fp8 normal range and the
   1/64 compensation is folded into the psum-evacuation scales.
 - softmax batched per 2-head group: scores accumulate into [128,2,137]
   f32 psum tiles (mask pre-added via one eye-matmul per tile), ONE exp
   ACT per tile (no accum-register reads), one grouped tensor_reduce for
   the denominators, one reciprocal and one broadcast tensor_tensor for
   the normalization of all 8 heads at once.
 - ACT table thrash eliminated: all LayerNorm Sqrt ops are emitted before
   the first Exp, each table loads exactly once, off the critical path.
 - LN apply moved to ACT (Identity with per-partition scale/bias), attn
   softmax chain on DVE, residual adds on GpSimd except the last chunk
   (fused psum-scale+residual scalar_tensor_tensor on DVE).
"""

import os

os.environ.setdefault("JAX_PLATFORMS", "axon")

from contextlib import ExitStack

import numpy as np
import ml_dtypes

import concourse.bass as bass
import concourse.bacc as bacc
import concourse.tile as tile
from concourse import mybir
from concourse.bass import broadcast_tensor_aps
from concourse.bass_utils import run_bass_kernel_spmd

F32 = mybir.dt.float32
F16 = mybir.dt.float16
F8 = mybir.dt.float8e4
FP8NP = ml_dtypes.float8_e4m3fn

B, T, D = 2, 2048, 512
H, DH = 8, 64
WF, WB = 6, 3
LN_EPS = 1e-3
NCORES = 8
TLOC = T // NCORES          # 256 queries per core
TIN = WF + TLOC + WB        # 265 local tokens incl. halo
NTT = 3                     # token tiles per batch (128+128+9)
NQC = 2                     # query chunks of 128 per batch
S = 128 + WF + WB           # 137 keys per query chunk
NEG = -30000.0              # additive mask value (fp16-safe)
WS = 64.0                   # host-side fp8 weight scale
DR = mybir.MatmulPerfMode.DoubleRow
USE_DR = os.environ.get("K_DR", "1") == "1"
BATCH_MASK = os.environ.get("K_BATCHMASK", "0") == "1"

_CACHE = {}


def _build_program():
    nc = bacc.Bacc(
        "TRN2", target_bir_lowering=False, debug=False, num_devices=NCORES
    )

    xs = nc.dram_tensor("xs", [B, TIN, D], F32, kind="ExternalInput").ap()
    w8d = nc.dram_tensor("w8", [128, 8, 2, D], F8, kind="ExternalInput").ap()
    maskd = nc.dram_tensor("maskd", [128, NQC, 2, S], F16, kind="ExternalInput").ap()
    eye16d = nc.dram_tensor("eye16", [128, 128], F16, kind="ExternalInput").ap()
    xq32d = nc.dram_tensor("xq32", [128, B * NQC, D], F32, kind="ExternalInput").ap()
    outd = nc.dram_tensor("out", [B, TLOC, D], F32, kind="ExternalOutput").ap()

    with tile.TileContext(nc) as tc, ExitStack() as ctx:
        _emit(ctx, tc, xs, w8d, maskd, eye16d, xq32d, outd)

    nc.compile()
    return nc


def _emit(ctx, tc, xs, w8d, maskd, eye16d, xq32d, outd):
    nc = tc.nc
    EXP = mybir.ActivationFunctionType.Exp
    SQRT = mybir.ActivationFunctionType.Sqrt
    COPY = mybir.ActivationFunctionType.Copy
    IDENT = mybir.ActivationFunctionType.Identity
    SUB = mybir.AluOpType.subtract
    MULT = mybir.AluOpType.mult
    ADD = mybir.AluOpType.add

    consts = ctx.enter_context(tc.tile_pool(name="consts", bufs=1))
    persist = ctx.enter_context(tc.tile_pool(name="persist", bufs=1))
    ln_tmp = ctx.enter_context(tc.tile_pool(name="ln_tmp", bufs=3))
    attn_tmp = ctx.enter_context(tc.tile_pool(name="attn_tmp", bufs=2))
    # PSUM budget 8 banks: sc(3) proj(2) atm(1) att(1) ctx2(1)
    ps_scx = ctx.enter_context(tc.tile_pool(name="ps_scx", bufs=3, space="PSUM"))
    ps_proj = ctx.enter_context(tc.tile_pool(name="ps_proj", bufs=2, space="PSUM"))
    ps_at = ctx.enter_context(tc.tile_pool(name="ps_at", bufs=1, space="PSUM"))
    ps_att = ctx.enter_context(tc.tile_pool(name="ps_att", bufs=1, space="PSUM"))
    ps_ctx = ctx.enter_context(tc.tile_pool(name="ps_ctx", bufs=1, space="PSUM"))

    # ---- constants / weights (DMA issue spread across idle queues) ----------
    eye16 = consts.tile([128, 128], F16)
    nc.sync.dma_start(eye16, eye16d)
    mask_sb = consts.tile([128, NQC, 2, S], F16)
    nc.sync.dma_start(mask_sb, maskd)
    w_sb = consts.tile([128, 8, 2, D], F8)
    nc.sync.dma_start(w_sb[:, 0:4, :, :], w8d[:, 0:4, :, :])
    nc.gpsimd.dma_start(w_sb[:, 4:8, :, :], w8d[:, 4:8, :, :])
    xq32 = consts.tile([128, B * NQC, D], F32)
    nc.gpsimd.dma_start(xq32, xq32d)
    epst = consts.tile([128, 1], F32)
    nc.vector.memset(epst, LN_EPS)
    # load the Sqrt ACT table immediately (1.3us, during the DMA prologue)
    warm = consts.tile([128, 1], F32)
    nc.scalar.activation(out=warm, in_=epst, func=SQRT, bias=epst)

    # ---- persistent tiles ---------------------------------------------------
    x_sb = persist.tile([128, 2 * NTT, D], F32)
    xtr = persist.tile([128, 4, 2 * 384], F8)    # [dpart, dchunk, b*384+tok]
    q_sb = persist.tile([128, 4, B, TLOC], F16, tag="q_sb")
    k_sb = persist.tile([128, 4, B, TIN], F16, tag="k_sb")
    vt_sb = persist.tile([128, B, NTT, D], F16, tag="vt_sb")
    out_stage = persist.tile([128, B * NQC, D], F32, tag="out_stage")

    for b in range(B):
        nc.gpsimd.memset(x_sb[:, b * NTT + 2, :], 0.0)
    # per-tile x loads so LayerNorm can start on tile 0 early
    for b in range(B):
        eng = nc.scalar if b == 0 else nc.sync
        for i in range(2):
            eng.dma_start(
                x_sb[:, b * NTT + i, :], xs[b, 128 * i : 128 * (i + 1), :]
            )
        eng.dma_start(x_sb[:9, b * NTT + 2, :], xs[b, 256:TIN, :])

    mvall = persist.tile([128, 2 * NTT, 2], F32, tag="mvall")
    rstd_all = persist.tile([128, 2 * NTT], F32, tag="rstd_all")
    nmr_all = persist.tile([128, 2 * NTT], F32, tag="nmr_all")

    def emit_ln_stats(b, i):
        xt = x_sb[:, b * NTT + i, :]
        st = ln_tmp.tile([128, 6], F32, tag="st")
        nc.vector.bn_stats(out=st, in_=xt)
        nc.vector.bn_aggr(out=mvall[:, b * NTT + i, :], in_=st)

    sd_tiles = {}

    def emit_ln_rstd(b):
        # one Sqrt ACT per batch: the only Sqrt-table ops in the program
        sl = slice(b * NTT, b * NTT + NTT)
        sd = ln_tmp.tile([128, NTT], F32, tag="sd_all", name="sd")
        sd_tiles[b] = sd
        nc.scalar.activation(out=sd, in_=mvall[:, sl, 1], func=SQRT, bias=epst)
        nc.vector.reciprocal(out=rstd_all[:, sl], in_=sd)
        # nmr = -mean * rstd (per-tile per-partition LN bias)
        nc.vector.scalar_tensor_tensor(
            out=nmr_all[:, sl], in0=mvall[:, sl, 0], scalar=-1.0,
            in1=rstd_all[:, sl], op0=MULT, op1=MULT,
        )

    def emit_ln_apply(b, i):
        xt = x_sb[:, b * NTT + i, :]
        idx = b * NTT + i
        xr = ln_tmp.tile([128, D], F16, tag="xr")
        nc.scalar.activation(
            out=xr, in_=xt, func=IDENT,
            bias=nmr_all[:, idx : idx + 1], scale=rstd_all[:, idx : idx + 1],
        )
        pt = ps_scx.tile([128, 4, 128], F16, tag="sc")
        for j in range(4):
            nc.tensor.transpose(pt[:, j, :], xr[:, 128 * j : 128 * j + 128], eye16)
        dst = xtr[:, :, 384 * b + 128 * i : 384 * b + 128 * (i + 1)]
        if i % 2 == 0:
            nc.vector.tensor_copy(dst, pt)
        else:
            nc.scalar.activation(out=dst, in_=pt, func=COPY)

    def w(widx, t):
        # [128, 2, D] fp8 DoubleRow pair t of matrix widx (0=q,1=k,2=v,3=o)
        return w_sb[:, 2 * widx + t, :, :]

    def emit_proj_qk(b):
        # q: queries only (N=256)
        for hkt in range(4):
            pp = ps_proj.tile([128, D], F32, tag="proj")
            for t in range(2):
                if USE_DR:
                    nc.tensor.matmul(
                        pp[:, 0:TLOC],
                        w(0, t)[:, :, 128 * hkt : 128 * (hkt + 1)],
                        xtr[:, 2 * t : 2 * t + 2, 384 * b + WF : 384 * b + WF + TLOC],
                        start=(t == 0), stop=(t == 1), perf_mode=DR,
                    )
                else:
                    for i2 in range(2):
                        nc.tensor.matmul(
                            pp[:, 0:TLOC],
                            w(0, t)[:, i2, 128 * hkt : 128 * (hkt + 1)],
                            xtr[:, 2 * t + i2, 384 * b + WF : 384 * b + WF + TLOC],
                            start=(t == 0 and i2 == 0), stop=(t == 1 and i2 == 1),
                        )
            if hkt % 2 == 0:
                nc.vector.tensor_scalar(
                    out=q_sb[:, hkt, b, :], in0=pp[:, 0:TLOC],
                    scalar1=1.0 / WS, scalar2=None, op0=MULT,
                )
            else:
                nc.scalar.activation(
                    out=q_sb[:, hkt, b, :], in_=pp[:, 0:TLOC],
                    func=COPY, scale=1.0 / WS,
                )
        # k incl. halo (N=265)
        for hkt in range(4):
            pp = ps_proj.tile([128, D], F32, tag="proj")
            for t in range(2):
                if USE_DR:
                    nc.tensor.matmul(
                        pp[:, 0:TIN],
                        w(1, t)[:, :, 128 * hkt : 128 * (hkt + 1)],
                        xtr[:, 2 * t : 2 * t + 2, 384 * b : 384 * b + TIN],
                        start=(t == 0), stop=(t == 1), perf_mode=DR,
                    )
                else:
                    for i2 in range(2):
                        nc.tensor.matmul(
                            pp[:, 0:TIN],
                            w(1, t)[:, i2, 128 * hkt : 128 * (hkt + 1)],
                            xtr[:, 2 * t + i2, 384 * b : 384 * b + TIN],
                            start=(t == 0 and i2 == 0), stop=(t == 1 and i2 == 1),
                        )
            nc.scalar.activation(
                out=k_sb[:, hkt, b, :], in_=pp[:, 0:TIN],
                func=COPY, scale=1.0 / WS,
            )

    def emit_proj_v(b):
        # vT per token tile (N=512)
        for i in range(NTT):
            pp = ps_proj.tile([128, D], F32, tag="proj")
            for t in range(2):
                if USE_DR:
                    nc.tensor.matmul(
                        pp,
                        xtr[:, 2 * t : 2 * t + 2, 384 * b + 128 * i : 384 * b + 128 * (i + 1)],
                        w(2, t),
                        start=(t == 0), stop=(t == 1), perf_mode=DR,
                    )
                else:
                    for i2 in range(2):
                        nc.tensor.matmul(
                            pp,
                            xtr[:, 2 * t + i2, 384 * b + 128 * i : 384 * b + 128 * (i + 1)],
                            w(2, t)[:, i2, :],
                            start=(t == 0 and i2 == 0), stop=(t == 1 and i2 == 1),
                        )
            if i % 2 == 0:
                nc.scalar.activation(
                    out=vt_sb[:, b, i, :], in_=pp, func=COPY, scale=1.0 / WS
                )
            else:
                nc.vector.tensor_scalar(
                    out=vt_sb[:, b, i, :], in0=pp,
                    scalar1=1.0 / WS, scalar2=None, op0=MULT,
                )

    def emit_attn_a(b, cq, ea):
        """Scores + exp for all 8 heads of one query chunk."""
        q0 = 128 * cq
        s0 = 128 * cq
        for j in range(4):
            sc = ps_scx.tile([128, 2, S], F32, tag="sc")
            if BATCH_MASK:
                nc.tensor.matmul(
                    sc, eye16, mask_sb[:, cq, :, :],
                    start=True, stop=False, skip_group_check=True,
                )
            for u in range(2):
                hp = 64 * u
                if not BATCH_MASK:
                    nc.tensor.matmul(
                        sc[:, u, :], eye16, mask_sb[:, cq, u, :],
                        start=True, stop=False,
                    )
                nc.tensor.matmul(
                    sc[:, u, :],
                    q_sb[hp : hp + 64, j, b, q0 : q0 + 128],
                    k_sb[hp : hp + 64, j, b, s0 : s0 + S],
                    start=False, stop=True, skip_group_check=BATCH_MASK,
                )
            nc.scalar.activation(
                out=ea[:, 2 * j : 2 * j + 2, :], in_=sc, func=EXP, scale=0.125
            )

    def emit_attn_b(b, cq, ea, last):
        """Softmax normalize + ctx + out-proj + residual + store."""
        sums = attn_tmp.tile([128, 8], F32, tag="sums")
        nc.vector.reduce_sum(out=sums, in_=ea, axis=mybir.AxisListType.X)
        rec = attn_tmp.tile([128, 8, 1], F32, tag="rec")
        nc.vector.reciprocal(out=rec[:, :, 0], in_=sums)
        ean = attn_tmp.tile([128, 8, S], F16, tag="ean")
        for h in range(8):
            nc.vector.tensor_scalar(
                out=ean[:, h, :], in0=ea[:, h, :],
                scalar1=rec[:, h, :], scalar2=None, op0=MULT,
            )

        at_m = ps_at.tile([128, 8, 128], F16, tag="atm")
        at_t = ps_att.tile([9, 8, 128], F16, tag="att")
        for h in range(8):
            nc.tensor.transpose(at_m[:, h, :], ean[:, h, 0:128], eye16)
        for h in range(8):
            nc.tensor.transpose(at_t[:, h, :], ean[:, h, 128:S], eye16)
        atm_sb = attn_tmp.tile([128, 8, 128], F16, tag="atm_sb")
        att_sb = attn_tmp.tile([9, 8, 128], F16, tag="att_sb")
        nc.vector.tensor_copy(atm_sb, at_m)
        nc.vector.tensor_copy(att_sb, at_t)

        ctx2 = ps_ctx.tile([128, 4, 128], F32, tag="ctx2")
        for h in range(8):
            hp = 64 * (h % 2)
            hkt = h // 2
            nc.tensor.matmul(
                ctx2[hp : hp + 64, hkt, :],
                vt_sb[:, b, cq, 64 * h : 64 * h + 64],
                atm_sb[:, h, :],
                start=True, stop=False,
            )
            nc.tensor.matmul(
                ctx2[hp : hp + 64, hkt, :],
                vt_sb[0:9, b, cq + 1, 64 * h : 64 * h + 64],
                att_sb[0:9, h, :],
                start=False, stop=True,
            )
        ctxn = attn_tmp.tile([128, 4, 128], F8, tag="ctxn")
        nc.scalar.activation(out=ctxn, in_=ctx2, func=COPY)

        op = ps_proj.tile([128, D], F32, tag="proj")
        for t in range(2):
            if USE_DR:
                nc.tensor.matmul(
                    op, ctxn[:, 2 * t : 2 * t + 2, :], w(3, t),
                    start=(t == 0), stop=(t == 1), perf_mode=DR,
                )
            else:
                for i2 in range(2):
                    nc.tensor.matmul(
                        op, ctxn[:, 2 * t + i2, :], w(3, t)[:, i2, :],
                        start=(t == 0 and i2 == 0), stop=(t == 1 and i2 == 1),
                    )
        idx = b * NQC + cq
        oslot = out_stage[:, idx, :]
        if last:
            nc.vector.scalar_tensor_tensor(
                out=oslot, in0=op, scalar=1.0 / WS, in1=xq32[:, idx, :],
                op0=MULT, op1=ADD,
            )
        else:
            nc.scalar.activation(out=oslot, in_=op, func=COPY, scale=1.0 / WS)
            nc.gpsimd.tensor_add(oslot, oslot, xq32[:, idx, :])
        nc.sync.dma_start(outd[b, 128 * cq : 128 * (cq + 1), :], oslot)

    # ---- schedule -----------------------------------------------------------
    for i in range(NTT):
        emit_ln_stats(0, i)
    emit_ln_rstd(0)
    for i in range(NTT):
        emit_ln_apply(0, i)
    emit_proj_qk(0)
    for i in range(NTT):
        emit_ln_stats(1, i)
    emit_ln_rstd(1)
    # Exp table load depends on the LAST Sqrt so the scheduler cannot hoist
    # it earlier (which would thrash the ACT table back and forth)
    warm2 = consts.tile([128, 1], F32)
    nc.scalar.activation(out=warm2, in_=sd_tiles[1][:, 0:1], func=EXP)
    for i in range(NTT):
        emit_ln_apply(1, i)
    emit_proj_v(0)

    ea_pool = {}
    for key in [(0, 0), (0, 1), (1, 0), (1, 1)]:
        ea_pool[key] = attn_tmp.tile([128, 8, S], F16, tag="ea", name="ea")

    emit_attn_a(0, 0, ea_pool[(0, 0)])
    emit_proj_qk(1)
    emit_attn_b(0, 0, ea_pool[(0, 0)], last=False)
    emit_attn_a(0, 1, ea_pool[(0, 1)])
    emit_proj_v(1)
    emit_attn_b(0, 1, ea_pool[(0, 1)], last=False)
    emit_attn_a(1, 0, ea_pool[(1, 0)])
    emit_attn_b(1, 0, ea_pool[(1, 0)], last=False)
    emit_attn_a(1, 1, ea_pool[(1, 1)])
    emit_attn_b(1, 1, ea_pool[(1, 1)], last=True)


def _dr_pack(W):
    """[D, M] -> [2, 128, 2, M] DoubleRow k-tile pairs."""
    W4 = W.reshape(4, 128, -1)
    return np.stack(
        [np.stack([W4[2 * t], W4[2 * t + 1]], axis=1) for t in range(2)]
    )


def _prep_host(inputs):
    """Host-side weight folding and per-core slicing."""
    x = np.asarray(inputs["x"], np.float32)
    gamma = np.asarray(inputs["gamma"], np.float32)
    beta = np.asarray(inputs["beta"], np.float32)
    Wq = np.asarray(inputs["Wq"], np.float32).reshape(D, H * DH)
    Wk = np.asarray(inputs["Wk"], np.float32).reshape(D, H * DH)
    Wv = np.asarray(inputs["Wv"], np.float32).reshape(D, H * DH)
    Wo = np.asarray(inputs["Wo"], np.float32).reshape(H * DH, D)
    bq = np.asarray(inputs["bq"], np.float32).reshape(H * DH)
    bk = np.asarray(inputs["bk"], np.float32).reshape(H * DH)
    bv = np.asarray(inputs["bv"], np.float32).reshape(H * DH)
    bo = np.asarray(inputs["bo"], np.float32).reshape(D)

    Wq2 = gamma[:, None] * Wq
    Wk2 = gamma[:, None] * Wk
    Wv2 = gamma[:, None] * Wv
    cq = bq + beta @ Wq
    ck = bk + beta @ Wk
    cv = bv + beta @ Wv
    if np.any(cq) or np.any(ck):
        raise NotImplementedError("nonzero q/k bias not supported")
    bo_eff = bo + cv @ Wo

    w8 = np.concatenate(
        [_dr_pack(WS * m) for m in (Wq2, Wk2, Wv2, Wo)], axis=0
    ).astype(FP8NP)
    # device layout [p, m, i, d] so the weight DMA is contiguous per partition
    w8 = np.ascontiguousarray(w8.transpose(1, 0, 2, 3))

    eye16 = np.eye(128, dtype=np.float16)

    in_maps = []
    for c in range(NCORES):
        g0 = TLOC * c - WF
        xs = np.zeros((B, TIN, D), np.float32)
        lo, hi = max(0, g0), min(T, g0 + TIN)
        xs[:, lo - g0 : hi - g0, :] = x[:, lo:hi, :]

        mask = np.full((NQC, 128, S), NEG, np.float16)
        for cqi in range(NQC):
            r = np.arange(128)[:, None]
            sl = np.arange(S)[None, :]
            gj = g0 + 128 * cqi + sl
            valid = (sl - r >= 0) & (sl - r <= WF + WB) & (gj >= 0) & (gj < T)
            mask[cqi][valid] = 0.0
        mask2 = np.ascontiguousarray(
            np.repeat(mask[:, :, None, :], 2, axis=2).transpose(1, 0, 2, 3)
        )

        xq32 = np.ascontiguousarray(
            x[:, TLOC * c : TLOC * (c + 1), :]
            .reshape(B, NQC, 128, D)
            .transpose(2, 0, 1, 3)
            .reshape(128, B * NQC, D)
        )
        in_maps.append(
            {
                "xs": xs, "w8": w8, "maskd": mask2,
                "eye16": eye16, "xq32": xq32,
            }
        )
    return in_maps, bo_eff


def kernel(**inputs) -> np.ndarray:
    if "nc" not in _CACHE:
        _CACHE["nc"] = _build_program()
    nc = _CACHE["nc"]
    in_maps, bo_eff = _prep_host(inputs)
    res = run_bass_kernel_spmd(nc, in_maps, list(range(NCORES)))
    out = np.empty((B, T, D), np.float32)
    for c in range(NCORES):
        out[:, TLOC * c : TLOC * (c + 1), :] = res.results[c]["out"]
    if np.any(bo_eff):
        out += bo_eff
    return out
